# revision 5
# baseline (speedup 1.0000x reference)
"""Trainium2 Bass kernel for nn_CorrectedHistogramLoss.

Math: soft triangular histogram of R=64 bins over N=1M clamped similarities,
for sim and dissim1; then cumsum/dot scalar finalize (host, f64).

Identity: in z-space (z = 31.5*(x+1) in [0, 63], bin width 1, integer
thresholds k) the triangular hat is the second finite difference of the
cumulative threshold family

  S_k = sum_n relu(z_n - k),   hist_r * N = S_{r-1} - 2 S_r + S_{r+1}

with free endpoints S_{-1} = Z + N, S_0 = Z (= sum z), S_63 = S_64 = 0.
Only S_1..S_62 and Z need measuring: 63 accumulation passes per array.

Exact on-host family conversions let every engine use its cheapest op:
  DVE / GPSIMD max:  M_k = sum max(z,k) = kN + S_k
  DVE / GPSIMD min:  W_k = sum min(z,k) = Z - S_k   (W_63 = Z)
  ACT Relu+bias:     S_k directly

Sharding: one ARRAY per half of the mesh - cores 0-3 process sim, cores 4-7
process dissim1, each as a [128, 2048] fp16 tile (262144 elements).  Halving
the pass count at double length amortizes fixed per-pass overheads (measured
CoreSim costs at F=2048: DVE fp16 578ns in 4x perf mode, GPSIMD 1789ns, ACT
2174ns) -> balanced split DVE 40 / GPSIMD 13 / ACT 10 passes ~= 23us.

z is precomputed on host in fp16 (exact min/max vs integer thresholds;
<=2^-11 relative position error, far below the 2e-2 gate).  All accumulators
are fp32; per-core column sums are combined on host in f64.
"""

import sys

sys.path.insert(0, "/opt/trn_rl_repo")

import numpy as np

import concourse.bass as bass
import concourse.bacc as bacc
import concourse.mybir as mybir
import concourse.tile as tile
from concourse.bass_utils import run_bass_kernel_spmd

# ---------------------------------------------------------------- constants
N = 1_048_576
R = 64
PLOSS = 0.1
NCORES = 8
P, F = 128, 2048
CORES_PER_ARRAY = 4  # cores 0-3: sim, cores 4-7: dissim1

# Threshold k ranges per engine (S-equivalents k in [1, 62] plus Z=W_63).
# GPSIMD is unusable: walrus rejects TensorScalarPtr on the Pool engine.
ACT_KS = list(range(25, 38))            # S_k directly          (13 passes)
GPS_KS = []
DVE_M_KS = list(range(1, 25))           # M_k = kN + S_k        (24 passes)
DVE_W_KS = list(range(38, 64))          # W_k (incl. Z = W_63)  (26 passes)

PLAN = (
    [("dve", "M", k) for k in DVE_M_KS]
    + [("dve", "W", k) for k in DVE_W_KS]
    + [("gps", "W", k) for k in GPS_KS]
    + [("act", "S", k) for k in ACT_KS]
)
COUNTS = {
    "dve": len(DVE_M_KS) + len(DVE_W_KS),
    "act": len(ACT_KS),
}
if GPS_KS:
    COUNTS["gps"] = len(GPS_KS)
N_ACT_THR = len(ACT_KS)


# ------------------------------------------------------------- bass program
def build_program():
    nc = bacc.Bacc(
        "TRN2",
        target_bir_lowering=False,
        debug=False,
        num_devices=NCORES,
    )
    z = nc.declare_dram_parameter("z", [P, F], mybir.dt.float16, isOutput=False)
    cb = nc.declare_dram_parameter(
        "cb", [P, N_ACT_THR], mybir.dt.float32, isOutput=False
    )
    ncols = sum(COUNTS.values())
    acc_out = nc.declare_dram_parameter(
        "acc", [P, ncols], mybir.dt.float32, isOutput=True
    )

    with tile.TileContext(nc) as tc:
        with (
            tc.tile_pool(name="data", bufs=1) as data_pool,
            tc.tile_pool(name="trash", bufs=2) as trash_pool,
            tc.tile_pool(name="accs", bufs=1) as acc_pool,
        ):
            z_t = data_pool.tile([P, F], mybir.dt.float16, tag="z", name="z_t")
            nc.sync.dma_start(z_t[:], z[:])
            cb_t = data_pool.tile(
                [P, N_ACT_THR], mybir.dt.float32, tag="cb", name="cbt"
            )
            nc.sync.dma_start(cb_t[:], cb[:])

            acc_t = acc_pool.tile(
                [P, sum(COUNTS.values())], mybir.dt.float32, tag="acc",
                name="acc_t",
            )
            base = {}
            off = 0
            for eng, cnt in COUNTS.items():
                base[eng] = off
                off += cnt
            trash = {
                eng: [
                    trash_pool.tile(
                        [P, F], mybir.dt.float16, tag=f"t_{eng}", name=f"t{eng}{i}"
                    )
                    for i in range(2)
                ]
                for eng in COUNTS
            }

            cols = {eng: 0 for eng in COUNTS}
            for eng, fam, k in PLAN:
                col = cols[eng]
                cols[eng] += 1
                tr = trash[eng][col % 2]
                gcol = base[eng] + col
                out_col = acc_t[:, gcol : gcol + 1]
                if eng == "act":
                    ci = k - ACT_KS[0]
                    nc.scalar.activation(
                        tr[:], z_t[:], mybir.ActivationFunctionType.Relu,
                        bias=cb_t[:, ci : ci + 1], scale=1.0, accum_out=out_col,
                    )
                else:
                    op0 = (
                        mybir.AluOpType.max if fam == "M" else mybir.AluOpType.min
                    )
                    engine = nc.vector if eng == "dve" else nc.gpsimd
                    engine.tensor_scalar(
                        tr[:], z_t[:], float(k), None,
                        op0=op0, op1=mybir.AluOpType.add,
                        accum_out=out_col,
                    )

            nc.sync.dma_start(acc_out[:], acc_t[:])

    nc.compile()
    return nc


_PROGRAM = None


def _get_program():
    global _PROGRAM
    if _PROGRAM is None:
        _PROGRAM = build_program()
    return _PROGRAM


# ------------------------------------------------------------------ driver
def _bias_table():
    cb = np.zeros((P, N_ACT_THR), dtype=np.float32)
    for ci, k in enumerate(ACT_KS):
        cb[:, ci] = -np.float32(k)
    return cb


def _to_z(x):
    x = np.asarray(x, dtype=np.float32)
    z = 31.5 * (np.clip(x, -1.0, 1.0) + 1.0)
    z16 = z.astype(np.float16)
    np.clip(z16, np.float16(0.0), np.float16(63.0), out=z16)
    return np.ascontiguousarray(z16).reshape(CORES_PER_ARRAY, P, F)


def run_device(sim, dissim1, trace=False):
    """Run the SPMD kernel; returns (S, results) where S[a] is a float64
    array of S-equivalents S[a][k] for k in 0..64 (S_0=Z etc. filled in)."""
    shards = [_to_z(sim), _to_z(dissim1)]
    cb = _bias_table()
    nc = _get_program()
    in_maps = [
        {"z": shards[i // CORES_PER_ARRAY][i % CORES_PER_ARRAY], "cb": cb}
        for i in range(NCORES)
    ]
    res = run_bass_kernel_spmd(nc, in_maps, list(range(NCORES)), trace=trace)

    S = [np.zeros(65, dtype=np.float64) for _ in range(2)]
    for a in range(2):
        cores = range(a * CORES_PER_ARRAY, (a + 1) * CORES_PER_ARRAY)
        tot = sum(
            res.results[c]["acc"].astype(np.float64).sum(axis=0) for c in cores
        )
        base = {}
        off = 0
        for eng, cnt in COUNTS.items():
            base[eng] = off
            off += cnt
        cols = {eng: 0 for eng in COUNTS}
        vals = {}
        for eng, fam, k in PLAN:
            vals[(fam, k)] = tot[base[eng] + cols[eng]]
            cols[eng] += 1
        Z = vals[("W", 63)]
        for (fam, k), v in vals.items():
            if fam == "S":
                S[a][k] = v
            elif fam == "M":
                S[a][k] = v - float(k) * N
            elif k < 63:
                S[a][k] = Z - v
        S[a][0] = Z  # S_0 = Z; S_63 = S_64 = 0 already
    return S, res


def _hist_from_S(S_a):
    h = np.empty(R, dtype=np.float64)
    Z = S_a[0]
    for r in range(R):
        sm1 = Z + float(N) if r == 0 else S_a[r - 1]
        h[r] = sm1 - 2.0 * S_a[r] + S_a[r + 1]
    return h / N


def finalize(hp, hm):
    hp_c, hm_c = np.cumsum(hp), np.cumsum(hm)
    q = 1.0 - PLOSS
    num = (
        q * q * np.dot(hp_c, hm)
        - q * PLOSS * np.dot(hp_c, hp)
        - q * PLOSS * np.dot(hm_c, hm)
        + PLOSS * PLOSS * np.dot(hm_c, hp)
    )
    return num / (1.0 - 4.0 * PLOSS + 4.0 * PLOSS * PLOSS)


def kernel(sim, dissim1, dissim2=None, margin=None, anchor_swap=None, **_kw):
    S, _ = run_device(sim, dissim1, trace=False)
    return np.float32(finalize(_hist_from_S(S[0]), _hist_from_S(S[1])))


# revision 6
# speedup vs baseline: 1.0146x; 1.0146x over previous
"""Trainium2 Bass kernel for nn_CorrectedHistogramLoss.

Math: soft triangular histogram of R=64 bins over N=1M clamped similarities,
for sim and dissim1; then cumsum/dot scalar finalize (host, f64).

Identity: in z-space (z = 31.5*(x+1) in [0, 63], bin width 1, integer
thresholds k) the triangular hat is the second finite difference of the
cumulative threshold family

  S_k = sum_n relu(z_n - k),   hist_r * N = S_{r-1} - 2 S_r + S_{r+1}

with free endpoints S_{-1} = Z + N, S_0 = Z (= sum z), S_63 = S_64 = 0.
Only S_1..S_62 and Z need measuring: 63 accumulation passes per array.

Exact on-host family conversions let every engine use its cheapest op:
  DVE / GPSIMD max:  M_k = sum max(z,k) = kN + S_k
  DVE / GPSIMD min:  W_k = sum min(z,k) = Z - S_k   (W_63 = Z)
  ACT Relu+bias:     S_k directly

Sharding: one ARRAY per half of the mesh - cores 0-3 process sim, cores 4-7
process dissim1, each as a [128, 2048] fp16 tile (262144 elements).  Halving
the pass count at double length amortizes fixed per-pass overheads (measured
CoreSim costs at F=2048: DVE fp16 578ns in 4x perf mode, GPSIMD 1789ns, ACT
2174ns) -> balanced split DVE 40 / GPSIMD 13 / ACT 10 passes ~= 23us.

z is precomputed on host in fp16 (exact min/max vs integer thresholds;
<=2^-11 relative position error, far below the 2e-2 gate).  All accumulators
are fp32; per-core column sums are combined on host in f64.
"""

import sys

sys.path.insert(0, "/opt/trn_rl_repo")

import numpy as np

import concourse.bass as bass
import concourse.bacc as bacc
import concourse.mybir as mybir
import concourse.tile as tile
from concourse.bass_utils import run_bass_kernel_spmd

# ---------------------------------------------------------------- constants
N = 1_048_576
R = 64
PLOSS = 0.1
NCORES = 8
P, F = 128, 2048
CORES_PER_ARRAY = 4  # cores 0-3: sim, cores 4-7: dissim1

# Threshold k ranges per engine (S-equivalents k in [1, 62] plus Z=W_63).
# GPSIMD is unusable: walrus rejects TensorScalarPtr on the Pool engine.
ACT_KS = list(range(25, 38))            # S_k directly          (13 passes)
GPS_KS = []
DVE_M_KS = list(range(1, 25))           # M_k = kN + S_k        (24 passes)
DVE_W_KS = list(range(38, 64))          # W_k (incl. Z = W_63)  (26 passes)

PLAN = (
    [("dve", "M", k) for k in DVE_M_KS]
    + [("dve", "W", k) for k in DVE_W_KS]
    + [("gps", "W", k) for k in GPS_KS]
    + [("act", "S", k) for k in ACT_KS]
)
COUNTS = {
    "dve": len(DVE_M_KS) + len(DVE_W_KS),
    "act": len(ACT_KS),
}
if GPS_KS:
    COUNTS["gps"] = len(GPS_KS)
N_ACT_THR = len(ACT_KS)
SPLIT_N = 1  # first thresholds per engine computed as two half-tile passes


# ------------------------------------------------------------- bass program
def build_program():
    nc = bacc.Bacc(
        "TRN2",
        target_bir_lowering=False,
        debug=False,
        num_devices=NCORES,
    )
    z = nc.declare_dram_parameter("z", [P, F], mybir.dt.float16, isOutput=False)
    cb = nc.declare_dram_parameter(
        "cb", [P, N_ACT_THR], mybir.dt.float32, isOutput=False
    )
    ncols = sum(COUNTS.values()) + SPLIT_N * len(COUNTS)
    acc_out = nc.declare_dram_parameter(
        "acc", [P, ncols], mybir.dt.float32, isOutput=True
    )

    with tile.TileContext(nc) as tc:
        with (
            tc.tile_pool(name="data", bufs=1) as data_pool,
            tc.tile_pool(name="trash", bufs=2) as trash_pool,
            tc.tile_pool(name="accs", bufs=1) as acc_pool,
        ):
            z_t = data_pool.tile([P, F], mybir.dt.float16, tag="z", name="z_t")
            F2 = F // 2
            nc.sync.dma_start(z_t[:, :F2], z[:, :F2])
            nc.sync.dma_start(z_t[:, F2:], z[:, F2:])
            cb_t = data_pool.tile(
                [P, N_ACT_THR], mybir.dt.float32, tag="cb", name="cbt"
            )
            nc.sync.dma_start(cb_t[:], cb[:])

            acc_t = acc_pool.tile(
                [P, sum(COUNTS.values()) + SPLIT_N * len(COUNTS)],
                mybir.dt.float32, tag="acc", name="acc_t",
            )
            base = {}
            off = 0
            for eng, cnt in COUNTS.items():
                base[eng] = off
                off += cnt + SPLIT_N
            trash = {
                eng: [
                    trash_pool.tile(
                        [P, F], mybir.dt.float16, tag=f"t_{eng}", name=f"t{eng}{i}"
                    )
                    for i in range(2)
                ]
                for eng in COUNTS
            }

            cols = {eng: 0 for eng in COUNTS}
            nth = {eng: 0 for eng in COUNTS}
            for eng, fam, k in PLAN:
                # First threshold per engine runs as two half-tile passes so
                # compute starts after the first half of the z DMA lands.
                halves = (
                    [(0, F2), (F2, F)] if nth[eng] < SPLIT_N else [(0, F)]
                )
                nth[eng] += 1
                for lo, hi in halves:
                    col = cols[eng]
                    cols[eng] += 1
                    tr = trash[eng][col % 2]
                    gcol = base[eng] + col
                    out_col = acc_t[:, gcol : gcol + 1]
                    zi = z_t[:, lo:hi]
                    if eng == "act":
                        ci = k - ACT_KS[0]
                        nc.scalar.activation(
                            tr[:, lo:hi], zi, mybir.ActivationFunctionType.Relu,
                            bias=cb_t[:, ci : ci + 1], scale=1.0,
                            accum_out=out_col,
                        )
                    else:
                        op0 = (
                            mybir.AluOpType.max
                            if fam == "M"
                            else mybir.AluOpType.min
                        )
                        engine = nc.vector if eng == "dve" else nc.gpsimd
                        engine.tensor_scalar(
                            tr[:, lo:hi], zi, float(k), None,
                            op0=op0, op1=mybir.AluOpType.add,
                            accum_out=out_col,
                        )

            nc.sync.dma_start(acc_out[:], acc_t[:])

    nc.compile()
    return nc


_PROGRAM = None


def _get_program():
    global _PROGRAM
    if _PROGRAM is None:
        _PROGRAM = build_program()
    return _PROGRAM


# ------------------------------------------------------------------ driver
def _bias_table():
    cb = np.zeros((P, N_ACT_THR), dtype=np.float32)
    for ci, k in enumerate(ACT_KS):
        cb[:, ci] = -np.float32(k)
    return cb


def _to_z(x):
    x = np.asarray(x, dtype=np.float32)
    z = 31.5 * (np.clip(x, -1.0, 1.0) + 1.0)
    z16 = z.astype(np.float16)
    np.clip(z16, np.float16(0.0), np.float16(63.0), out=z16)
    return np.ascontiguousarray(z16).reshape(CORES_PER_ARRAY, P, F)


def run_device(sim, dissim1, trace=False):
    """Run the SPMD kernel; returns (S, results) where S[a] is a float64
    array of S-equivalents S[a][k] for k in 0..64 (S_0=Z etc. filled in)."""
    shards = [_to_z(sim), _to_z(dissim1)]
    cb = _bias_table()
    nc = _get_program()
    in_maps = [
        {"z": shards[i // CORES_PER_ARRAY][i % CORES_PER_ARRAY], "cb": cb}
        for i in range(NCORES)
    ]
    res = run_bass_kernel_spmd(nc, in_maps, list(range(NCORES)), trace=trace)

    S = [np.zeros(65, dtype=np.float64) for _ in range(2)]
    for a in range(2):
        cores = range(a * CORES_PER_ARRAY, (a + 1) * CORES_PER_ARRAY)
        tot = sum(
            res.results[c]["acc"].astype(np.float64).sum(axis=0) for c in cores
        )
        base = {}
        off = 0
        for eng, cnt in COUNTS.items():
            base[eng] = off
            off += cnt + SPLIT_N
        cols = {eng: 0 for eng in COUNTS}
        nth = {eng: 0 for eng in COUNTS}
        vals = {}
        for eng, fam, k in PLAN:
            c = base[eng] + cols[eng]
            if nth[eng] < SPLIT_N:
                vals[(fam, k)] = tot[c] + tot[c + 1]
                cols[eng] += 2
            else:
                vals[(fam, k)] = tot[c]
                cols[eng] += 1
            nth[eng] += 1
        Z = vals[("W", 63)]
        for (fam, k), v in vals.items():
            if fam == "S":
                S[a][k] = v
            elif fam == "M":
                S[a][k] = v - float(k) * N
            elif k < 63:
                S[a][k] = Z - v
        S[a][0] = Z  # S_0 = Z; S_63 = S_64 = 0 already
    return S, res


def _hist_from_S(S_a):
    h = np.empty(R, dtype=np.float64)
    Z = S_a[0]
    for r in range(R):
        sm1 = Z + float(N) if r == 0 else S_a[r - 1]
        h[r] = sm1 - 2.0 * S_a[r] + S_a[r + 1]
    return h / N


def finalize(hp, hm):
    hp_c, hm_c = np.cumsum(hp), np.cumsum(hm)
    q = 1.0 - PLOSS
    num = (
        q * q * np.dot(hp_c, hm)
        - q * PLOSS * np.dot(hp_c, hp)
        - q * PLOSS * np.dot(hm_c, hm)
        + PLOSS * PLOSS * np.dot(hm_c, hp)
    )
    return num / (1.0 - 4.0 * PLOSS + 4.0 * PLOSS * PLOSS)


def kernel(sim, dissim1, dissim2=None, margin=None, anchor_swap=None, **_kw):
    S, _ = run_device(sim, dissim1, trace=False)
    return np.float32(finalize(_hist_from_S(S[0]), _hist_from_S(S[1])))
